# revision 9
# baseline (speedup 1.0000x reference)
"""DynaLoRALinear Trainium2 kernel.

Data-parallel over batch B across 8 NeuronCores (one sample per core).
Per core:
  - router:  logits = pooled @ (W_r @ gating_W).T  computed as a sharded
    partial (each core contracts over a 512-wide slice of D) + AllReduce.
  - gate weights from expert_scores ranks + module_prob>0.5 branch select.
  - base:    out = x_b @ W_base.T + b_base   (tf32 matmuls, fp32 PSUM accum)
  - lora:    t = x_b @ A_cat.T (fused into chunk-0 k-loop), then
             out += t @ (B_cat * gate).T
Matmuls use float32r (tf32) operands pre-rounded on host: 1 cyc/row on PE
(4x faster than fp32) at ~3e-4 scale-relative absmax error.
"""

import sys
import types

import numpy as np

B, L, D, E, R, NMOD = 8, 2048, 4096, 4, 8, 7
N_CORES = 8
DSH = D // N_CORES  # 512: per-core slice of D for the router shard
ER = E * R          # 32
O_C = 512           # W_base column chunk (double-buffered in SBUF)
N_CHUNK = D // O_C  # 4
KT = D // 128       # 32 k-tiles
XB = 8              # k-tiles batched per x DMA
MT = L // 128       # 16 m-tiles


def _round_tf32(a) -> np.ndarray:
    """Round-to-nearest-even fp32 -> tf32 (10-bit mantissa), keep fp32 bits."""
    a = np.ascontiguousarray(a, dtype=np.float32)
    u = a.view(np.uint32).astype(np.uint64)
    u = (u + 0xFFF + ((u >> 13) & 1)) & 0xFFFFE000
    return np.ascontiguousarray(u.astype(np.uint32)).view(np.float32)


def _install_profile_hook():
    """Make bass_utils' trace path importable (no-op if already present)."""
    try:
        import antenv.axon_hooks  # noqa: F401
        return
    except ImportError:
        pass
    try:
        import antenv
    except ImportError:
        return
    mod = types.ModuleType("antenv.axon_hooks")
    mod._hook = None
    mod.set_axon_ntff_profile_hook = lambda h: setattr(mod, "_hook", h)
    mod.get_axon_ntff_profile_hook = lambda: mod._hook
    sys.modules["antenv.axon_hooks"] = mod
    antenv.axon_hooks = mod
    try:
        from trn_agent_boot.trn_boot import _ntff_profile_via_ctypes
        hook = _ntff_profile_via_ctypes("/opt/axon/libaxon_pjrt.so")
        if hook is not None:
            mod.set_axon_ntff_profile_hook(hook)
    except Exception:
        pass


_PROGRAM_CACHE = {}


def _build_program(k: int, module_idx: int):
    import concourse.mybir as mybir
    import concourse.tile as tile
    from concourse import bacc
    from concourse.masks import make_identity

    f32 = mybir.dt.float32
    f32r = mybir.dt.float32r
    alu = mybir.AluOpType
    act_fn = mybir.ActivationFunctionType

    k_lo = max(1, k // 2)

    nc = bacc.Bacc("TRN2", target_bir_lowering=False, debug=False,
                   num_devices=N_CORES)

    # --- DRAM I/O -------------------------------------------------------
    xT = nc.dram_tensor("xT", [D, L], f32r, kind="ExternalInput")
    WbT = nc.dram_tensor("WbT", [D, D], f32r, kind="ExternalInput")
    gw = nc.dram_tensor("gw", [D, DSH], f32r, kind="ExternalInput")
    WrT = nc.dram_tensor("WrT", [D, NMOD], f32r, kind="ExternalInput")
    pooledT = nc.dram_tensor("pooledT", [DSH, B], f32, kind="ExternalInput")
    scores_f = nc.dram_tensor("scores_f", [1, E * B], f32,
                              kind="ExternalInput")
    A_rhs = nc.dram_tensor("A_rhs", [D, ER], f32r, kind="ExternalInput")
    B_cat = nc.dram_tensor("B_cat", [ER, D], f32, kind="ExternalInput")
    b_row = nc.dram_tensor("b_row", [1, D], f32, kind="ExternalInput")
    msel = nc.dram_tensor("msel", [ER, E * B], f32, kind="ExternalInput")
    out = nc.dram_tensor("out", [L, D], f32, kind="ExternalOutput")

    with tile.TileContext(nc) as tc:
        with (
            tc.tile_pool(name="const", bufs=1) as const_pool,
            tc.tile_pool(name="gatep", bufs=1) as gate_pool,
        ):
            ident = const_pool.tile([128, 128], f32)
            make_identity(nc, ident)
            gate32 = gate_pool.tile([ER, 1], f32)

            # ============== router =====================================
            with (
                tc.tile_pool(name="rsb", bufs=1) as rsb,
                tc.tile_pool(name="rgw", bufs=4) as rgw,
                tc.tile_pool(name="rps", bufs=1, space="PSUM") as rps,
                tc.tile_pool(name="rdram", bufs=1, space="DRAM") as rdram,
            ):
                # r1: W_comb_part [NMOD, DSH] = WrT.T @ gw  (tf32)
                wr_sb = rsb.tile([128, KT, NMOD], f32r)
                nc.sync.dma_start(
                    wr_sb[:], WrT[:].rearrange("(a p) m -> p a m", p=128))
                wc_ps = rps.tile([NMOD, DSH], f32)
                for kt in range(KT):
                    gwt = rgw.tile([128, DSH], f32r, tag="gwt",
                                   name=f"gwt_{kt}")
                    nc.sync.dma_start(gwt[:], gw[kt * 128:(kt + 1) * 128, :])
                    nc.tensor.matmul(wc_ps[:],
                                     wr_sb[:, kt, :],
                                     gwt[:], start=(kt == 0),
                                     stop=(kt == KT - 1))
                wc_sb = rsb.tile([NMOD, DSH], f32)
                nc.vector.tensor_copy(wc_sb[:], wc_ps[:])

                # transpose W_comb_part -> [DSH, NMOD] (4x PE transpose)
                wct = rsb.tile([128, 4 * NMOD], f32)
                for j in range(4):
                    tp = rps.tile([128, NMOD], f32, tag="tp", name=f"tp_{j}")
                    nc.tensor.transpose(
                        tp[:], wc_sb[:, j * 128:(j + 1) * 128],
                        ident[0:NMOD, 0:NMOD])
                    nc.vector.tensor_copy(
                        wct[:, j * NMOD:(j + 1) * NMOD], tp[:])

                # r2: partial logits [NMOD, B] (fp32)
                pt_sb = rsb.tile([128, 4, B], f32)
                nc.sync.dma_start(
                    pt_sb[:],
                    pooledT[:].rearrange("(a p) m -> p a m", p=128))
                lg_ps = rps.tile([NMOD, B], f32)
                for j in range(4):
                    nc.tensor.matmul(lg_ps[:],
                                     wct[:, j * NMOD:(j + 1) * NMOD],
                                     pt_sb[:, j, :],
                                     start=(j == 0), stop=(j == 3))
                lp_sb = rsb.tile([NMOD, B], f32)
                nc.vector.tensor_copy(lp_sb[:], lg_ps[:])

                # AllReduce partial logits across the 8 cores
                cc_in = rdram.tile([NMOD, B], f32)
                cc_out = rdram.tile([NMOD, B], f32)
                nc.gpsimd.dma_start(cc_in[:], lp_sb[:])
                nc.gpsimd.collective_compute(
                    "AllReduce", alu.add,
                    replica_groups=[list(range(N_CORES))],
                    ins=[cc_in.opt()], outs=[cc_out.opt()])
                lg_sb = rsb.tile([NMOD, B], f32)
                nc.gpsimd.dma_start(lg_sb[:], cc_out[:])

                # logits.T [B, NMOD] via PE transpose
                ltp = rps.tile([B, NMOD], f32)
                nc.tensor.transpose(ltp[:], lg_sb[:], ident[0:NMOD, 0:NMOD])
                lt = rsb.tile([B, NMOD], f32)
                nc.vector.tensor_copy(lt[:], ltp[:])

                # softmax -> p0 -> hi = (p0 > 0.5)
                mx = rsb.tile([B, 1], f32)
                nc.vector.tensor_reduce(out=mx[:], in_=lt[:], op=alu.max,
                                        axis=mybir.AxisListType.X)
                mxn = rsb.tile([B, 1], f32)
                nc.vector.tensor_scalar_mul(mxn[:], mx[:], -1.0)
                ex = rsb.tile([B, NMOD], f32)
                nc.scalar.activation(ex[:], lt[:], act_fn.Exp, bias=mxn[:])
                sm = rsb.tile([B, 1], f32)
                nc.vector.tensor_reduce(out=sm[:], in_=ex[:], op=alu.add,
                                        axis=mybir.AxisListType.X)
                rs = rsb.tile([B, 1], f32)
                nc.vector.reciprocal(rs[:], sm[:])
                p0 = rsb.tile([B, 1], f32)
                nc.vector.tensor_mul(
                    p0[:], ex[:, module_idx:module_idx + 1], rs[:])
                hi = rsb.tile([B, 1], f32)
                nc.vector.tensor_single_scalar(hi[:], p0[:], 0.5, alu.is_gt)
                hp = rps.tile([1, B], f32)
                nc.tensor.transpose(hp[:], hi[:], ident[0:B, 0:B])
                hi_row = rsb.tile([1, B], f32)
                nc.vector.tensor_copy(hi_row[:], hp[:])

                # expert ranks on scores [1, E*B] (exact fp32 compares,
                # all on partition 0: partition slicing needs 32-alignment)
                sc = rsb.tile([1, E * B], f32)
                nc.sync.dma_start(sc[:], scores_f[:])
                rank = rsb.tile([1, E * B], f32)
                nc.vector.memset(rank[:], 0.0)
                tmp = rsb.tile([1, B], f32)
                for e in range(E):
                    re = rank[:, e * B:(e + 1) * B]
                    se = sc[:, e * B:(e + 1) * B]
                    for e2 in range(E):
                        if e2 == e:
                            continue
                        s2 = sc[:, e2 * B:(e2 + 1) * B]
                        nc.vector.tensor_tensor(tmp[:], s2, se, op=alu.is_gt)
                        nc.vector.tensor_add(re, re, tmp[:])
                        if e2 < e:
                            nc.vector.tensor_tensor(tmp[:], s2, se,
                                                    op=alu.is_equal)
                            nc.vector.tensor_add(re, re, tmp[:])

                w_hi = rsb.tile([1, E * B], f32)
                nc.vector.tensor_scalar(w_hi[:], rank[:], float(k),
                                        1.0 / float(k),
                                        op0=alu.is_lt, op1=alu.mult)
                w_lo = rsb.tile([1, E * B], f32)
                nc.vector.tensor_scalar(w_lo[:], rank[:], float(k_lo),
                                        1.0 / float(k_lo),
                                        op0=alu.is_lt, op1=alu.mult)
                # gate = w_lo + hi * (w_hi - w_lo); hi_row [1, B] repeats per e
                diff = rsb.tile([1, E * B], f32)
                nc.vector.tensor_sub(diff[:], w_hi[:], w_lo[:])
                gate = rsb.tile([1, E * B], f32)
                for e in range(E):
                    nc.vector.tensor_mul(gate[:, e * B:(e + 1) * B],
                                         diff[:, e * B:(e + 1) * B],
                                         hi_row[:])
                nc.vector.tensor_add(gate[:], gate[:], w_lo[:])

                # gate32[p] = gate[e(p), core_id]: broadcast to 32 partitions,
                # mask with per-core host constant, reduce along free dim.
                gateb = rsb.tile([ER, E * B], f32)
                nc.gpsimd.partition_broadcast(gateb[:], gate[:])
                msel_sb = rsb.tile([ER, E * B], f32)
                nc.sync.dma_start(msel_sb[:], msel[:])
                g32m = rsb.tile([ER, E * B], f32)
                nc.vector.tensor_tensor(g32m[:], gateb[:], msel_sb[:],
                                        op=alu.mult)
                nc.vector.tensor_reduce(out=gate32[:], in_=g32m[:],
                                        op=alu.add,
                                        axis=mybir.AxisListType.X)

            # ============== main: base + lora ==========================
            with (
                tc.tile_pool(name="wpool", bufs=2 * KT + 4) as wpool,
                tc.tile_pool(name="xpool", bufs=3) as xpool,
                tc.tile_pool(name="apool", bufs=1) as apool,
                tc.tile_pool(name="tpool", bufs=MT) as tpool,
                tc.tile_pool(name="tstg", bufs=2) as tstg,
                tc.tile_pool(name="bpool", bufs=1) as bpool,
                tc.tile_pool(name="biasp", bufs=1) as biasp,
                tc.tile_pool(name="epool", bufs=3) as epool,
                tc.tile_pool(name="mps", bufs=6, space="PSUM") as mps,
                tc.tile_pool(name="tps", bufs=1, space="PSUM") as tps,
            ):
                a_sb = apool.tile([128, KT, ER], f32r)
                nc.sync.dma_start(
                    a_sb[:], A_rhs[:].rearrange("(a p) m -> p a m", p=128))

                tT_tiles = [None] * MT
                for c in range(N_CHUNK):
                    col0 = c * O_C
                    wtiles = []
                    for kt in range(KT):
                        wt = wpool.tile([128, O_C], f32r, tag="w",
                                        name=f"w_{c}_{kt}")
                        nc.sync.dma_start(
                            wt[:],
                            WbT[kt * 128:(kt + 1) * 128, col0:col0 + O_C])
                        wtiles.append(wt)
                    # B chunk scaled by this core's gate (tf32-rounded out)
                    b_stg = bpool.tile([ER, O_C], f32, tag="bstg",
                                       name=f"bstg_{c}")
                    nc.sync.dma_start(b_stg[:], B_cat[:, col0:col0 + O_C])
                    b_scl = bpool.tile([ER, O_C], f32r, tag="bscl",
                                       name=f"bscl_{c}")
                    nc.vector.tensor_scalar_mul(b_scl[:], b_stg[:],
                                                gate32[:, 0:1])
                    brow_c = biasp.tile([1, O_C], f32, tag="brow",
                                        name=f"brow_{c}")
                    nc.sync.dma_start(brow_c[:], b_row[:, col0:col0 + O_C])
                    bias_bc = biasp.tile([128, O_C], f32, tag="biasbc",
                                         name=f"biasbc_{c}")
                    nc.gpsimd.partition_broadcast(bias_bc[:], brow_c[:])

                    for m in range(MT):
                        ps = mps.tile([128, O_C], f32, tag="ps",
                                      name=f"ps_{c}_{m}")
                        if c == 0:
                            pt = tps.tile([128, ER], f32, tag="pt",
                                          name=f"pt_{m}")
                        for kb in range(KT // XB):
                            xt = xpool.tile([128, XB, 128], f32r, tag="x",
                                            name=f"x_{c}_{m}_{kb}")
                            nc.sync.dma_start(
                                xt[:],
                                xT[kb * XB * 128:(kb + 1) * XB * 128,
                                   m * 128:(m + 1) * 128]
                                .rearrange("(a p) f -> p a f", p=128))
                            for j in range(XB):
                                kt = kb * XB + j
                                nc.tensor.matmul(ps[:], xt[:, j, :],
                                                 wtiles[kt][:],
                                                 start=(kt == 0), stop=False)
                                if c == 0:
                                    nc.tensor.matmul(
                                        pt[:], xt[:, j, :],
                                        a_sb[:, kt, :],
                                        start=(kt == 0),
                                        stop=(kt == KT - 1))
                        if c == 0:
                            t_sb = tstg.tile([128, ER], f32, tag="tsb",
                                             name=f"tsb_{m}")
                            nc.vector.tensor_copy(t_sb[:], pt[:])
                            ptT = tps.tile([ER, 128], f32, tag="ptT",
                                           name=f"ptT_{m}")
                            nc.tensor.transpose(ptT[:], t_sb[:], ident[:])
                            tT = tpool.tile([ER, 128], f32r, tag="tT",
                                            name=f"tT_{m}")
                            nc.vector.tensor_copy(tT[:], ptT[:])
                            tT_tiles[m] = tT
                        # lora accumulate into the same psum block
                        nc.tensor.matmul(ps[:], tT_tiles[m][:], b_scl[:],
                                         start=False, stop=True)
                        # evict with bias add
                        ev = epool.tile([128, O_C], f32, tag="ev",
                                        name=f"ev_{c}_{m}")
                        nc.vector.tensor_add(ev[:], ps[:], bias_bc[:])
                        nc.sync.dma_start(
                            out[m * 128:(m + 1) * 128, col0:col0 + O_C],
                            ev[:])

    nc.compile()
    return nc


def kernel(**inputs) -> np.ndarray:
    _install_profile_hook()

    x = np.asarray(inputs["x"], dtype=np.float32)
    expert_scores = np.asarray(inputs["expert_scores"], dtype=np.float32)
    W_base = np.asarray(inputs["W_base"], dtype=np.float32)
    b_base = np.asarray(inputs["b_base"], dtype=np.float32)
    gating_W = np.asarray(inputs["gating_W"], dtype=np.float32)
    W_r = np.asarray(inputs["W_r"], dtype=np.float32)
    lora_A = np.asarray(inputs["lora_A"], dtype=np.float32)
    lora_B = np.asarray(inputs["lora_B"], dtype=np.float32)
    module_idx = int(np.asarray(inputs["module_idx"]))
    k = int(np.asarray(inputs["k"]))

    key = (k, module_idx)
    if key not in _PROGRAM_CACHE:
        _PROGRAM_CACHE[key] = _build_program(k, module_idx)
    nc = _PROGRAM_CACHE[key]

    # --- host-side layout prep (transposes/slices/rounding only) --------
    WbT_np = _round_tf32(W_base.T)                       # [D, D]
    WrT_np = _round_tf32(W_r.T)                          # [D, NMOD]
    A_np = _round_tf32(lora_A.reshape(ER, D).T)          # [D, ER]
    B_np = np.ascontiguousarray(
        lora_B.transpose(0, 2, 1).reshape(ER, D))        # [ER, D] fp32
    scores_f_np = np.ascontiguousarray(
        expert_scores.T.reshape(1, E * B))               # [1, E*B]
    b_row_np = b_base.reshape(1, D)
    pooled = x[:, -1, :]                                 # [B, D]

    in_maps = []
    for c in range(N_CORES):
        msel_np = np.zeros((ER, E, B), dtype=np.float32)
        for p in range(ER):
            msel_np[p, p // R, c] = 1.0
        msel_np = msel_np.reshape(ER, E * B)
        in_maps.append({
            "xT": _round_tf32(x[c].T),
            "WbT": WbT_np,
            "gw": _round_tf32(gating_W[:, c * DSH:(c + 1) * DSH]),
            "WrT": WrT_np,
            "pooledT": np.ascontiguousarray(
                pooled[:, c * DSH:(c + 1) * DSH].T),
            "scores_f": scores_f_np,
            "A_rhs": A_np,
            "B_cat": B_np,
            "b_row": b_row_np,
            "msel": msel_np,
        })

    from concourse.bass_utils import run_bass_kernel_spmd
    res = run_bass_kernel_spmd(nc, in_maps, core_ids=list(range(N_CORES)))
    return np.stack([res.results[c]["out"] for c in range(N_CORES)], axis=0)


if __name__ == "__main__":
    rng = np.random.default_rng(0)
    demo = {
        "x": (rng.standard_normal((B, L, D)) * 0.02).astype(np.float32),
        "expert_scores": rng.random((B, E), dtype=np.float32),
        "W_base": (rng.standard_normal((D, D)) * 0.02).astype(np.float32),
        "b_base": np.zeros(D, np.float32),
        "gating_W": (rng.standard_normal((D, D)) * 0.02).astype(np.float32),
        "W_r": (rng.standard_normal((NMOD, D)) * 0.02).astype(np.float32),
        "lora_A": (rng.standard_normal((E, R, D)) * 0.02).astype(np.float32),
        "lora_B": (rng.standard_normal((E, D, R)) * 0.02).astype(np.float32),
        "module_idx": 0,
        "k": 2,
    }
    y = kernel(**demo)
    print("out", y.shape, y.dtype, float(np.abs(y).max()))


# revision 10
# speedup vs baseline: 1.1126x; 1.1126x over previous
"""DynaLoRALinear Trainium2 kernel.

Data-parallel over batch B across 8 NeuronCores (one sample per core).
Per core:
  - router:  logits = pooled @ (W_r @ gating_W).T  computed as a sharded
    partial (each core contracts over a 512-wide slice of D) + AllReduce.
  - gate weights from expert_scores ranks + module_prob>0.5 branch select.
  - base:    out = x_b @ W_base.T + b_base   (tf32 matmuls, fp32 PSUM accum)
  - lora:    t = x_b @ A_cat.T (fused into chunk-0 k-loop), then
             out += t @ (B_cat * gate).T
Matmuls use float32r (tf32) operands pre-rounded on host: 1 cyc/row on PE
(4x faster than fp32) at ~3e-4 scale-relative absmax error.
"""

import sys
import types

import numpy as np

B, L, D, E, R, NMOD = 8, 2048, 4096, 4, 8, 7
N_CORES = 8
DSH = D // N_CORES  # 512: per-core slice of D for the router shard
ER = E * R          # 32
O_C = 1024          # W_base column chunk cached in SBUF
N_CHUNK = D // O_C  # 4
KT = D // 128       # 32 k-tiles
XB = 8              # k-tiles batched per x DMA
MT = L // 128       # 16 m-tiles


def _round_tf32(a) -> np.ndarray:
    """Round-to-nearest-even fp32 -> tf32 (10-bit mantissa), keep fp32 bits."""
    a = np.ascontiguousarray(a, dtype=np.float32)
    u = a.view(np.uint32).astype(np.uint64)
    u = (u + 0xFFF + ((u >> 13) & 1)) & 0xFFFFE000
    return np.ascontiguousarray(u.astype(np.uint32)).view(np.float32)


def _install_profile_hook():
    """Make bass_utils' trace path importable (no-op if already present)."""
    try:
        import antenv.axon_hooks  # noqa: F401
        return
    except ImportError:
        pass
    try:
        import antenv
    except ImportError:
        return
    mod = types.ModuleType("antenv.axon_hooks")
    mod._hook = None
    mod.set_axon_ntff_profile_hook = lambda h: setattr(mod, "_hook", h)
    mod.get_axon_ntff_profile_hook = lambda: mod._hook
    sys.modules["antenv.axon_hooks"] = mod
    antenv.axon_hooks = mod
    try:
        from trn_agent_boot.trn_boot import _ntff_profile_via_ctypes
        hook = _ntff_profile_via_ctypes("/opt/axon/libaxon_pjrt.so")
        if hook is not None:
            mod.set_axon_ntff_profile_hook(hook)
    except Exception:
        pass


_PROGRAM_CACHE = {}


def _build_program(k: int, module_idx: int):
    import concourse.mybir as mybir
    import concourse.tile as tile
    from concourse import bacc
    from concourse.masks import make_identity

    f32 = mybir.dt.float32
    f32r = mybir.dt.float32r
    alu = mybir.AluOpType
    act_fn = mybir.ActivationFunctionType

    k_lo = max(1, k // 2)

    nc = bacc.Bacc("TRN2", target_bir_lowering=False, debug=False,
                   num_devices=N_CORES)

    # --- DRAM I/O -------------------------------------------------------
    xT = nc.dram_tensor("xT", [D, L], f32r, kind="ExternalInput")
    WbT = nc.dram_tensor("WbT", [D, D], f32r, kind="ExternalInput")
    gw = nc.dram_tensor("gw", [D, DSH], f32r, kind="ExternalInput")
    WrT = nc.dram_tensor("WrT", [D, NMOD], f32r, kind="ExternalInput")
    pooledT = nc.dram_tensor("pooledT", [DSH, B], f32, kind="ExternalInput")
    scores_f = nc.dram_tensor("scores_f", [1, E * B], f32,
                              kind="ExternalInput")
    A_rhs = nc.dram_tensor("A_rhs", [D, ER], f32r, kind="ExternalInput")
    B_cat = nc.dram_tensor("B_cat", [ER, D], f32, kind="ExternalInput")
    b_row = nc.dram_tensor("b_row", [1, D], f32, kind="ExternalInput")
    msel = nc.dram_tensor("msel", [ER, E * B], f32, kind="ExternalInput")
    out = nc.dram_tensor("out", [L, D], f32, kind="ExternalOutput")

    with tile.TileContext(nc) as tc:
        with (
            tc.tile_pool(name="const", bufs=1) as const_pool,
            tc.tile_pool(name="gatep", bufs=1) as gate_pool,
        ):
            ident = const_pool.tile([128, 128], f32)
            make_identity(nc, ident)
            gate32 = gate_pool.tile([ER, 1], f32)

            # ============== router =====================================
            with (
                tc.tile_pool(name="rsb", bufs=1) as rsb,
                tc.tile_pool(name="rgw", bufs=4) as rgw,
                tc.tile_pool(name="rps", bufs=1, space="PSUM") as rps,
                tc.tile_pool(name="rdram", bufs=1, space="DRAM") as rdram,
            ):
                # r1: W_comb_part [NMOD, DSH] = WrT.T @ gw  (tf32)
                wr_sb = rsb.tile([128, KT, NMOD], f32r)
                nc.sync.dma_start(
                    wr_sb[:], WrT[:].rearrange("(a p) m -> p a m", p=128))
                wc_ps = rps.tile([NMOD, DSH], f32)
                for kt in range(KT):
                    gwt = rgw.tile([128, DSH], f32r, tag="gwt",
                                   name=f"gwt_{kt}")
                    nc.sync.dma_start(gwt[:], gw[kt * 128:(kt + 1) * 128, :])
                    nc.tensor.matmul(wc_ps[:],
                                     wr_sb[:, kt, :],
                                     gwt[:], start=(kt == 0),
                                     stop=(kt == KT - 1))
                wc_sb = rsb.tile([NMOD, DSH], f32)
                nc.vector.tensor_copy(wc_sb[:], wc_ps[:])

                # transpose W_comb_part -> [DSH, NMOD] (4x PE transpose)
                wct = rsb.tile([128, 4 * NMOD], f32)
                for j in range(4):
                    tp = rps.tile([128, NMOD], f32, tag="tp", name=f"tp_{j}")
                    nc.tensor.transpose(
                        tp[:], wc_sb[:, j * 128:(j + 1) * 128],
                        ident[0:NMOD, 0:NMOD])
                    nc.vector.tensor_copy(
                        wct[:, j * NMOD:(j + 1) * NMOD], tp[:])

                # r2: partial logits [NMOD, B] (fp32)
                pt_sb = rsb.tile([128, 4, B], f32)
                nc.sync.dma_start(
                    pt_sb[:],
                    pooledT[:].rearrange("(a p) m -> p a m", p=128))
                lg_ps = rps.tile([NMOD, B], f32)
                for j in range(4):
                    nc.tensor.matmul(lg_ps[:],
                                     wct[:, j * NMOD:(j + 1) * NMOD],
                                     pt_sb[:, j, :],
                                     start=(j == 0), stop=(j == 3))
                lp_sb = rsb.tile([NMOD, B], f32)
                nc.vector.tensor_copy(lp_sb[:], lg_ps[:])

                # AllReduce partial logits across the 8 cores
                cc_in = rdram.tile([NMOD, B], f32)
                cc_out = rdram.tile([NMOD, B], f32)
                nc.gpsimd.dma_start(cc_in[:], lp_sb[:])
                nc.gpsimd.collective_compute(
                    "AllReduce", alu.add,
                    replica_groups=[list(range(N_CORES))],
                    ins=[cc_in.opt()], outs=[cc_out.opt()])
                lg_sb = rsb.tile([NMOD, B], f32)
                nc.gpsimd.dma_start(lg_sb[:], cc_out[:])

                # logits.T [B, NMOD] via PE transpose
                ltp = rps.tile([B, NMOD], f32)
                nc.tensor.transpose(ltp[:], lg_sb[:], ident[0:NMOD, 0:NMOD])
                lt = rsb.tile([B, NMOD], f32)
                nc.vector.tensor_copy(lt[:], ltp[:])

                # softmax -> p0 -> hi = (p0 > 0.5)
                mx = rsb.tile([B, 1], f32)
                nc.vector.tensor_reduce(out=mx[:], in_=lt[:], op=alu.max,
                                        axis=mybir.AxisListType.X)
                mxn = rsb.tile([B, 1], f32)
                nc.vector.tensor_scalar_mul(mxn[:], mx[:], -1.0)
                ex = rsb.tile([B, NMOD], f32)
                nc.scalar.activation(ex[:], lt[:], act_fn.Exp, bias=mxn[:])
                sm = rsb.tile([B, 1], f32)
                nc.vector.tensor_reduce(out=sm[:], in_=ex[:], op=alu.add,
                                        axis=mybir.AxisListType.X)
                rs = rsb.tile([B, 1], f32)
                nc.vector.reciprocal(rs[:], sm[:])
                p0 = rsb.tile([B, 1], f32)
                nc.vector.tensor_mul(
                    p0[:], ex[:, module_idx:module_idx + 1], rs[:])
                hi = rsb.tile([B, 1], f32)
                nc.vector.tensor_single_scalar(hi[:], p0[:], 0.5, alu.is_gt)
                hp = rps.tile([1, B], f32)
                nc.tensor.transpose(hp[:], hi[:], ident[0:B, 0:B])
                hi_row = rsb.tile([1, B], f32)
                nc.vector.tensor_copy(hi_row[:], hp[:])

                # expert ranks on scores [1, E*B] (exact fp32 compares,
                # all on partition 0: partition slicing needs 32-alignment)
                sc = rsb.tile([1, E * B], f32)
                nc.sync.dma_start(sc[:], scores_f[:])
                rank = rsb.tile([1, E * B], f32)
                nc.vector.memset(rank[:], 0.0)
                tmp = rsb.tile([1, B], f32)
                for e in range(E):
                    re = rank[:, e * B:(e + 1) * B]
                    se = sc[:, e * B:(e + 1) * B]
                    for e2 in range(E):
                        if e2 == e:
                            continue
                        s2 = sc[:, e2 * B:(e2 + 1) * B]
                        nc.vector.tensor_tensor(tmp[:], s2, se, op=alu.is_gt)
                        nc.vector.tensor_add(re, re, tmp[:])
                        if e2 < e:
                            nc.vector.tensor_tensor(tmp[:], s2, se,
                                                    op=alu.is_equal)
                            nc.vector.tensor_add(re, re, tmp[:])

                w_hi = rsb.tile([1, E * B], f32)
                nc.vector.tensor_scalar(w_hi[:], rank[:], float(k),
                                        1.0 / float(k),
                                        op0=alu.is_lt, op1=alu.mult)
                w_lo = rsb.tile([1, E * B], f32)
                nc.vector.tensor_scalar(w_lo[:], rank[:], float(k_lo),
                                        1.0 / float(k_lo),
                                        op0=alu.is_lt, op1=alu.mult)
                # gate = w_lo + hi * (w_hi - w_lo); hi_row [1, B] repeats per e
                diff = rsb.tile([1, E * B], f32)
                nc.vector.tensor_sub(diff[:], w_hi[:], w_lo[:])
                gate = rsb.tile([1, E * B], f32)
                for e in range(E):
                    nc.vector.tensor_mul(gate[:, e * B:(e + 1) * B],
                                         diff[:, e * B:(e + 1) * B],
                                         hi_row[:])
                nc.vector.tensor_add(gate[:], gate[:], w_lo[:])

                # gate32[p] = gate[e(p), core_id]: broadcast to 32 partitions,
                # mask with per-core host constant, reduce along free dim.
                gateb = rsb.tile([ER, E * B], f32)
                nc.gpsimd.partition_broadcast(gateb[:], gate[:])
                msel_sb = rsb.tile([ER, E * B], f32)
                nc.sync.dma_start(msel_sb[:], msel[:])
                g32m = rsb.tile([ER, E * B], f32)
                nc.vector.tensor_tensor(g32m[:], gateb[:], msel_sb[:],
                                        op=alu.mult)
                nc.vector.tensor_reduce(out=gate32[:], in_=g32m[:],
                                        op=alu.add,
                                        axis=mybir.AxisListType.X)

            # ============== main: base + lora ==========================
            with (
                tc.tile_pool(name="wpool", bufs=KT + 6) as wpool,
                tc.tile_pool(name="xpool", bufs=2) as xpool,
                tc.tile_pool(name="apool", bufs=1) as apool,
                tc.tile_pool(name="tpool", bufs=MT) as tpool,
                tc.tile_pool(name="tstg", bufs=2) as tstg,
                tc.tile_pool(name="bpool", bufs=1) as bpool,
                tc.tile_pool(name="biasp", bufs=1) as biasp,
                tc.tile_pool(name="epool", bufs=2) as epool,
                tc.tile_pool(name="mps", bufs=3, space="PSUM") as mps,
                tc.tile_pool(name="tps", bufs=1, space="PSUM") as tps,
            ):
                a_sb = apool.tile([128, KT, ER], f32r)
                nc.sync.dma_start(
                    a_sb[:], A_rhs[:].rearrange("(a p) m -> p a m", p=128))

                tT_tiles = [None] * MT
                for c in range(N_CHUNK):
                    col0 = c * O_C
                    wtiles = []
                    for kt in range(KT):
                        wt = wpool.tile([128, O_C], f32r, tag="w",
                                        name=f"w_{c}_{kt}")
                        nc.sync.dma_start(
                            wt[:],
                            WbT[kt * 128:(kt + 1) * 128, col0:col0 + O_C])
                        wtiles.append(wt)
                    # B chunk scaled by this core's gate (tf32-rounded out)
                    b_stg = bpool.tile([ER, O_C], f32, tag="bstg",
                                       name=f"bstg_{c}")
                    nc.sync.dma_start(b_stg[:], B_cat[:, col0:col0 + O_C])
                    b_scl = bpool.tile([ER, O_C], f32r, tag="bscl",
                                       name=f"bscl_{c}")
                    nc.vector.tensor_scalar_mul(b_scl[:], b_stg[:],
                                                gate32[:, 0:1])
                    brow_c = biasp.tile([1, O_C], f32, tag="brow",
                                        name=f"brow_{c}")
                    nc.sync.dma_start(brow_c[:], b_row[:, col0:col0 + O_C])
                    bias_bc = biasp.tile([128, O_C], f32, tag="biasbc",
                                         name=f"biasbc_{c}")
                    nc.gpsimd.partition_broadcast(bias_bc[:], brow_c[:])

                    for m in range(MT):
                        ps = mps.tile([128, O_C], f32, tag="ps",
                                      name=f"ps_{c}_{m}")
                        if c == 0:
                            pt = tps.tile([128, ER], f32, tag="pt",
                                          name=f"pt_{m}")
                        for kb in range(KT // XB):
                            xt = xpool.tile([128, XB, 128], f32r, tag="x",
                                            name=f"x_{c}_{m}_{kb}")
                            nc.sync.dma_start(
                                xt[:],
                                xT[kb * XB * 128:(kb + 1) * XB * 128,
                                   m * 128:(m + 1) * 128]
                                .rearrange("(a p) f -> p a f", p=128))
                            for j in range(XB):
                                kt = kb * XB + j
                                nc.tensor.matmul(ps[:, 0:512], xt[:, j, :],
                                                 wtiles[kt][:, 0:512],
                                                 start=(kt == 0), stop=False)
                                nc.tensor.matmul(ps[:, 512:1024], xt[:, j, :],
                                                 wtiles[kt][:, 512:1024],
                                                 start=(kt == 0), stop=False)
                                if c == 0:
                                    nc.tensor.matmul(
                                        pt[:], xt[:, j, :],
                                        a_sb[:, kt, :],
                                        start=(kt == 0),
                                        stop=(kt == KT - 1))
                        if c == 0:
                            t_sb = tstg.tile([128, ER], f32, tag="tsb",
                                             name=f"tsb_{m}")
                            nc.vector.tensor_copy(t_sb[:], pt[:])
                            ptT = tps.tile([ER, 128], f32, tag="ptT",
                                           name=f"ptT_{m}")
                            nc.tensor.transpose(ptT[:], t_sb[:], ident[:])
                            tT = tpool.tile([ER, 128], f32r, tag="tT",
                                            name=f"tT_{m}")
                            nc.vector.tensor_copy(tT[:], ptT[:])
                            tT_tiles[m] = tT
                        # lora accumulate into the same psum block
                        nc.tensor.matmul(ps[:, 0:512], tT_tiles[m][:],
                                         b_scl[:, 0:512],
                                         start=False, stop=False)
                        nc.tensor.matmul(ps[:, 512:1024], tT_tiles[m][:],
                                         b_scl[:, 512:1024],
                                         start=False, stop=True)
                        # evict with bias add
                        ev = epool.tile([128, O_C], f32, tag="ev",
                                        name=f"ev_{c}_{m}")
                        nc.vector.tensor_add(ev[:], ps[:], bias_bc[:])
                        nc.sync.dma_start(
                            out[m * 128:(m + 1) * 128, col0:col0 + O_C],
                            ev[:])

    nc.compile()
    return nc


def kernel(**inputs) -> np.ndarray:
    _install_profile_hook()

    x = np.asarray(inputs["x"], dtype=np.float32)
    expert_scores = np.asarray(inputs["expert_scores"], dtype=np.float32)
    W_base = np.asarray(inputs["W_base"], dtype=np.float32)
    b_base = np.asarray(inputs["b_base"], dtype=np.float32)
    gating_W = np.asarray(inputs["gating_W"], dtype=np.float32)
    W_r = np.asarray(inputs["W_r"], dtype=np.float32)
    lora_A = np.asarray(inputs["lora_A"], dtype=np.float32)
    lora_B = np.asarray(inputs["lora_B"], dtype=np.float32)
    module_idx = int(np.asarray(inputs["module_idx"]))
    k = int(np.asarray(inputs["k"]))

    key = (k, module_idx)
    if key not in _PROGRAM_CACHE:
        _PROGRAM_CACHE[key] = _build_program(k, module_idx)
    nc = _PROGRAM_CACHE[key]

    # --- host-side layout prep (transposes/slices/rounding only) --------
    WbT_np = _round_tf32(W_base.T)                       # [D, D]
    WrT_np = _round_tf32(W_r.T)                          # [D, NMOD]
    A_np = _round_tf32(lora_A.reshape(ER, D).T)          # [D, ER]
    B_np = np.ascontiguousarray(
        lora_B.transpose(0, 2, 1).reshape(ER, D))        # [ER, D] fp32
    scores_f_np = np.ascontiguousarray(
        expert_scores.T.reshape(1, E * B))               # [1, E*B]
    b_row_np = b_base.reshape(1, D)
    pooled = x[:, -1, :]                                 # [B, D]

    in_maps = []
    for c in range(N_CORES):
        msel_np = np.zeros((ER, E, B), dtype=np.float32)
        for p in range(ER):
            msel_np[p, p // R, c] = 1.0
        msel_np = msel_np.reshape(ER, E * B)
        in_maps.append({
            "xT": _round_tf32(x[c].T),
            "WbT": WbT_np,
            "gw": _round_tf32(gating_W[:, c * DSH:(c + 1) * DSH]),
            "WrT": WrT_np,
            "pooledT": np.ascontiguousarray(
                pooled[:, c * DSH:(c + 1) * DSH].T),
            "scores_f": scores_f_np,
            "A_rhs": A_np,
            "B_cat": B_np,
            "b_row": b_row_np,
            "msel": msel_np,
        })

    from concourse.bass_utils import run_bass_kernel_spmd
    res = run_bass_kernel_spmd(nc, in_maps, core_ids=list(range(N_CORES)))
    return np.stack([res.results[c]["out"] for c in range(N_CORES)], axis=0)


if __name__ == "__main__":
    rng = np.random.default_rng(0)
    demo = {
        "x": (rng.standard_normal((B, L, D)) * 0.02).astype(np.float32),
        "expert_scores": rng.random((B, E), dtype=np.float32),
        "W_base": (rng.standard_normal((D, D)) * 0.02).astype(np.float32),
        "b_base": np.zeros(D, np.float32),
        "gating_W": (rng.standard_normal((D, D)) * 0.02).astype(np.float32),
        "W_r": (rng.standard_normal((NMOD, D)) * 0.02).astype(np.float32),
        "lora_A": (rng.standard_normal((E, R, D)) * 0.02).astype(np.float32),
        "lora_B": (rng.standard_normal((E, D, R)) * 0.02).astype(np.float32),
        "module_idx": 0,
        "k": 2,
    }
    y = kernel(**demo)
    print("out", y.shape, y.dtype, float(np.abs(y).max()))
